# revision 1
# baseline (speedup 1.0000x reference)
"""Trainium2 Bass kernel for nn_Bessel: out = i0e(z) * exp(z - 2a), z = 2a*sqrt((1+x@yT)/2), a=10.

Math: out = exp(z - 20) * i0e(z) = exp(z - 20 + ln i0e(z)).
With unit-norm rows, z = sqrt(200*c + 200) for c = x@yT in [-0.726, 0.816],
so z lies in [7.4, 19.1].  On that interval we evaluate the exact identity
  out = exp(t(z) - 20),   t(z) = z + ln i0e(z)
via a minimax fit of t(z) over a basis the engines give us for free:

  mode "beta" : t ~= A + B*z + C*z^2          (z^2 = 200c+200 comes free via c)
                max rel err ~5.0e-3
  mode "lnexp": t ~= A + B*z + D*ln z         (ln z = l/2, l = Ln(200c+200))
                max rel err ~1.4e-4, single ACT table set (ln+exp)

Matmul runs as a bf16x2 split (x = xh + xl, y = yh + yl):
  c = [xh;xl]^T-stacked @ [yh;yh] + xh @ yl   (drops xl*yl, ~1e-6 abs)
full fp32 PE matmul would be 4x slower; float32r loses ~8e-5 abs.

Per core (row-shard of x, y replicated):
  PE:  c into PSUM (two bf16 matmuls per 128x512 tile)
  ACT: z = Sqrt(200c+200) evacuating PSUM   (beta)  [or l = Ln(...) for lnexp]
  DVE: w = z + s*c  fused scalar_tensor_tensor      [or w = z + s*l]
  ACT: out = Exp(B*w + bias)   (exp batched per GROUP M-tiles to limit
       sqrt<->exp activation-table switches; enforced via add_dep_helper)
  DMA: out tile -> HBM

Measured on trn2 (8-core SPMD, differential For_i timing): ~195-215 us per
core makespan vs ~95-100 us HBM write roofline; L2 rel err 4.5e-3 (gate 2e-2).
Pipeline structure: psum tiles [128,1024] x 4 bufs (PE->ACT->DVE chain),
GROUP=3 M-tiles per ACT table phase, zw pool group+1 bufs, output DMA per
4096-col half-tile.
"""

import contextlib

import numpy as np

import concourse.bacc as bacc
import concourse.mybir as mybir
from concourse.tile import TileContext
from concourse.tile_autobufs import add_dep_helper
from concourse.bass_utils import run_bass_kernel_spmd

AF = mybir.ActivationFunctionType
OP = mybir.AluOpType
F32 = mybir.dt.float32
BF16 = mybir.dt.bfloat16
BFNP = mybir.dt.np(BF16)

N_CORES = 8
N_ROWS, M_COLS, DIM = 8192, 8192, 64
ROWS = N_ROWS // N_CORES          # 1024 rows of x per core
MTILES = ROWS // 128              # 8 partition tiles per core
PSUM_FD = 1024                    # 2 PSUM banks per psum tile
NCHUNK = M_COLS // PSUM_FD        # 4 psum tiles per M-tile
GROUP = 3                         # M-tiles per ACT-table phase (beta mode)

# minimax coefficients for t(z) = z + ln(i0e(z)) on z in [7.30, 19.20]
BETA_A, BETA_B, BETA_C = -1.36067207867, 0.913667220475, 0.00171853078443
BETA_STT_SCALAR = 200.0 * BETA_C / BETA_B          # multiplies c
BETA_EXP_SCALE = BETA_B
BETA_EXP_BIAS = BETA_A + 200.0 * BETA_C - 20.0

LN_A, LN_B, LN_D = -0.857855881732, 1.00110543921, -0.525542926199
LN_STT_SCALAR = LN_D / (2.0 * LN_B)                # multiplies l = ln(200c+200)
LN_EXP_SCALE = LN_B
LN_EXP_BIAS = LN_A - 20.0

MODE = "beta"

_cache = {}
DEBUG_MAP = {}


def _build(mode, group=GROUP, zw_bufs=None, exp_split=2, psum_fd=PSUM_FD, batch_dep=True, iters=1, ztmp_bufs=1, out_bf16=False, obf_bufs=2):
    nc = bacc.Bacc(None, target_bir_lowering=False)
    # xs = [xh; xl] stacked bf16 shard (2*DIM x ROWS); ys = [yh; yh]; ylo = yl
    xs_d = nc.dram_tensor("xs", [2 * DIM, ROWS], BF16, kind="ExternalInput")
    ys_d = nc.dram_tensor("ys", [2 * DIM, M_COLS], BF16, kind="ExternalInput")
    yl_d = nc.dram_tensor("yl", [DIM, M_COLS], BF16, kind="ExternalInput")
    out_d = nc.dram_tensor("out", [ROWS, M_COLS], BF16 if out_bf16 else F32, kind="ExternalOutput")

    if mode == "beta":
        stt_scalar, exp_scale, exp_bias = BETA_STT_SCALAR, BETA_EXP_SCALE, BETA_EXP_BIAS
        evac_func = AF.Sqrt
    else:
        stt_scalar, exp_scale, exp_bias = LN_STT_SCALAR, LN_EXP_SCALE, LN_EXP_BIAS
        evac_func = AF.Ln

    with TileContext(nc) as tc:
        with (
            tc.tile_pool(name="inp", bufs=1) as inp,
            tc.tile_pool(name="consts", bufs=1) as consts,
            tc.tile_pool(name="zw", bufs=zw_bufs or (group if out_bf16 else group + 1)) as zwpool,
            tc.tile_pool(name="obf", bufs=obf_bufs) as obfpool,
            tc.tile_pool(name="ztmp", bufs=ztmp_bufs) as ztmp,
            tc.tile_pool(name="psum", bufs=4096 // psum_fd, space="PSUM") as psum,
        ):
            xs = inp.tile([2 * DIM, ROWS], BF16)
            ys = inp.tile([2 * DIM, M_COLS], BF16)
            yl = inp.tile([DIM, M_COLS], BF16)
            nc.sync.dma_start(out=xs[:], in_=xs_d[:])
            for q in range(0, M_COLS, 2048):
                nc.sync.dma_start(out=ys[:, q:q + 2048], in_=ys_d[:, q:q + 2048])
                nc.sync.dma_start(out=yl[:, q:q + 2048], in_=yl_d[:, q:q + 2048])

            b200 = consts.tile([128, 1], F32)
            nc.gpsimd.memset(b200[:], 200.0)
            bexp = consts.tile([128, 1], F32)
            nc.gpsimd.memset(bexp[:], float(exp_bias))

            nchunk = M_COLS // psum_fd
            mtile_groups = [
                list(range(g, min(g + group, MTILES)))
                for g in range(0, MTILES, group)
            ]
            loop_cm = tc.For_i(0, iters) if iters > 1 else contextlib.nullcontext(0)
            with loop_cm as _i:
              for group in mtile_groups:
                  zw_tiles = {}
                  last_evac = None
                  for m in group:
                      zw = zwpool.tile([128, M_COLS], F32, tag="zw")
                      zw_tiles[m] = zw
                      zsep = None
                      if mode == "lnexp":
                          zsep = ztmp.tile([128, M_COLS], F32, tag="zsep")
                      msl = slice(m * 128, (m + 1) * 128)
                      for nb in range(nchunk):
                          pt = psum.tile([128, psum_fd], F32, tag="ps")
                          for j in range(psum_fd // 512):
                              col = nb * psum_fd + j * 512
                              csl = slice(col, col + 512)
                              nc.tensor.matmul(
                                  pt[:, j * 512:(j + 1) * 512],
                                  xs[:, msl], ys[:, csl],
                                  start=True, stop=False,
                              )
                              nc.tensor.matmul(
                                  pt[:, j * 512:(j + 1) * 512],
                                  xs[:DIM, msl], yl[:, csl],
                                  start=False, stop=True,
                              )
                          sl = slice(nb * psum_fd, (nb + 1) * psum_fd)
                          # evac: z (or l) = func(200*c + 200)
                          last_evac = nc.scalar.activation(
                              zw[:, sl], pt[:], evac_func, bias=b200[:], scale=200.0
                          )
                          if mode == "beta":
                              # w = z + stt_scalar * c   (c still in PSUM)
                              nc.vector.scalar_tensor_tensor(
                                  zw[:, sl], pt[:], stt_scalar, zw[:, sl],
                                  OP.mult, OP.add,
                              )
                      if mode == "lnexp":
                          # z = exp(0.5 * l)
                          nc.scalar.activation(zsep[:], zw[:], AF.Exp, bias=0.0, scale=0.5)
                          # w = z + stt_scalar * l
                          nc.vector.scalar_tensor_tensor(
                              zw[:], zw[:], stt_scalar, zsep[:], OP.mult, OP.add
                          )
                  for m in group:
                      zw = zw_tiles[m]
                      efd = M_COLS // exp_split
                      if out_bf16:
                          obf = obfpool.tile([128, M_COLS], BF16, tag="obf")
                      for e in range(exp_split):
                          esl = slice(e * efd, (e + 1) * efd)
                          etgt = obf[:, esl] if out_bf16 else zw[:, esl]
                          exp_inst = nc.scalar.activation(
                              etgt, zw[:, esl], AF.Exp,
                              bias=bexp[:], scale=float(exp_scale)
                          )
                          if mode == "beta" and batch_dep and last_evac is not None:
                              # keep all of this group's Sqrt evacs ahead of its
                              # Exps so only two ACT-table loads happen per group
                              add_dep_helper(
                                  exp_inst.ins, last_evac.ins, sync=False,
                                  reason="batch exp after group sqrt (table switch)",
                              )
                          nc.sync.dma_start(
                              out=out_d[m * 128:(m + 1) * 128, esl], in_=etgt
                          )

    nc.finalize()
    return nc


LAST_RESULTS = None


def _split_bf16(a):
    hi = a.astype(BFNP)
    lo = (a - hi.astype(np.float32)).astype(BFNP)
    return hi, lo


def kernel(x: np.ndarray, y: np.ndarray) -> np.ndarray:
    global LAST_RESULTS
    x = np.ascontiguousarray(x, dtype=np.float32)
    y = np.ascontiguousarray(y, dtype=np.float32)
    assert x.shape == (N_ROWS, DIM) and y.shape == (M_COLS, DIM)

    if MODE not in _cache:
        _cache[MODE] = _build(MODE)
    nc = _cache[MODE]

    yT = y.T
    yh, yl = _split_bf16(yT)
    ys = np.ascontiguousarray(np.concatenate([yh, yh], axis=0))
    yl = np.ascontiguousarray(yl)

    in_maps = []
    for i in range(N_CORES):
        xT = x[i * ROWS:(i + 1) * ROWS].T
        xh, xl = _split_bf16(xT)
        xstack = np.ascontiguousarray(np.concatenate([xh, xl], axis=0))
        in_maps.append({"xs": xstack, "ys": ys, "yl": yl})

    LAST_RESULTS = run_bass_kernel_spmd(nc, in_maps, list(range(N_CORES)))
    out = np.concatenate([r["out"] for r in LAST_RESULTS.results], axis=0)
    if out.dtype != np.float32:
        out = out.astype(np.float32)
    return out



# revision 2
# speedup vs baseline: 1.1236x; 1.1236x over previous
"""Trainium2 Bass kernel for nn_Bessel: out = i0e(z) * exp(z - 2a), z = 2a*sqrt((1+x@yT)/2), a=10.

Math: out = exp(z - 20 + ln i0e(z)) = exp(t(z) - 20), t(z) = z + ln i0e(z).
With unit-norm rows, z = sqrt(200*c + 200) for c = x@yT in [-0.726, 0.816],
so z lies in [7.4, 19.1].  Key identity used here ("sqs" mode):

  t(z) ~= A + B*sqrt(z^2 + delta)        max rel err 2.05e-3 on [7.2, 19.3]

Since z^2 = 200c + 200 is linear in c, the whole correction folds into the
Sqrt activation's free affine:  w = Sqrt(200*c + (200+delta)), out =
Exp(B*w + (A-20)).  That removes the DVE combine pass the previous version
needed for its quadratic fit, leaving exactly two ACT passes per element.

Schedule per core (row-shard of x, y replicated; no collectives):
  Phase A (sqrt table): per 128x2048 PSUM chunk: PE bf16x2 matmul pair
      -> ACT Sqrt evac direct to fp16 w tiles (all 8 M-tiles fit in SBUF)
  Phase B (exp table):  per M-tile: ACT Exp -> bf16 out tile -> DMA to HBM
  One table switch per phase (2 per iteration) instead of 6; fp16 w
  storage (16 MB) is what makes the single-switch schedule fit in SBUF.
  bf16 output halves the HBM write (16 MB/core, upcast to fp32 on host).

Matmul is the bf16x2 split (x = xh + xl, y = yh + yl):
  c = [xh;xl]^T-stacked @ [yh;yh] + xh @ yl   (drops xl*yl, ~1e-6 abs)

Error budget (numpy-simulated, same seed-0 inputs): L2 rel 4.4e-3
(fit 1.2e-3 + fp16 w 3.9e-3 + bf16 out 1.1e-3), gate 2e-2.
ACT busy model: 32 sqrt x (2048+352)/1.2 + 8 exp x (8192+352)/1.2
  + 2 table loads = 64.0 + 56.9 + 5.3 = 126us -> ~135us makespan target.
"""

import contextlib

import numpy as np

import concourse.bacc as bacc
import concourse.mybir as mybir
from concourse.tile import TileContext
from concourse.tile_autobufs import add_dep_helper
from concourse.bass_utils import run_bass_kernel_spmd

AF = mybir.ActivationFunctionType
F32 = mybir.dt.float32
F16 = mybir.dt.float16
BF16 = mybir.dt.bfloat16
BFNP = mybir.dt.np(BF16)

N_CORES = 8
N_ROWS, M_COLS, DIM = 8192, 8192, 64
ROWS = N_ROWS // N_CORES          # 1024 rows of x per core
MTILES = ROWS // 128              # 8 partition tiles per core
PSUM_FD = 2048                    # 4 PSUM banks per psum tile, 2 bufs

# minimax fit of t(z) = z + ln(i0e(z)) ~= A + B*sqrt(z^2 + delta), z in [7.2, 19.3]
SQS_A = -2.18471144825
SQS_B = 0.980389112036
SQS_D = 6.67629017188
SQ_BIAS = 200.0 + SQS_D           # Sqrt(200*c + SQ_BIAS)
EXP_SCALE = SQS_B
EXP_BIAS = SQS_A - 20.0

MODE = "sqs"

_cache = {}


def _build(mode, iters=1, psum_fd=PSUM_FD, exp_split=1, obf_bufs=2, mm_n=512):
    assert mode == "sqs"
    nc = bacc.Bacc(None, target_bir_lowering=False)
    # xs = [xh; xl] stacked bf16 shard (2*DIM x ROWS); ys = [yh; yh]; yl low bits
    xs_d = nc.dram_tensor("xs", [2 * DIM, ROWS], BF16, kind="ExternalInput")
    ys_d = nc.dram_tensor("ys", [2 * DIM, M_COLS], BF16, kind="ExternalInput")
    yl_d = nc.dram_tensor("yl", [DIM, M_COLS], BF16, kind="ExternalInput")
    out_d = nc.dram_tensor("out", [ROWS, M_COLS], BF16, kind="ExternalOutput")

    with TileContext(nc) as tc:
        with (
            tc.tile_pool(name="inp", bufs=1) as inp,
            tc.tile_pool(name="consts", bufs=1) as consts,
            tc.tile_pool(name="zw", bufs=MTILES) as zwpool,
            tc.tile_pool(name="obf", bufs=obf_bufs) as obfpool,
            tc.tile_pool(name="psum", bufs=4096 // psum_fd, space="PSUM") as psum,
        ):
            xs = inp.tile([2 * DIM, ROWS], BF16)
            ys = inp.tile([2 * DIM, M_COLS], BF16)
            yl = inp.tile([DIM, M_COLS], BF16)
            nc.sync.dma_start(out=xs[:], in_=xs_d[:])
            for q in range(0, M_COLS, 2048):
                nc.sync.dma_start(out=ys[:, q:q + 2048], in_=ys_d[:, q:q + 2048])
                nc.sync.dma_start(out=yl[:, q:q + 2048], in_=yl_d[:, q:q + 2048])

            bsq = consts.tile([128, 1], F32)
            nc.gpsimd.memset(bsq[:], float(SQ_BIAS))
            bexp = consts.tile([128, 1], F32)
            nc.gpsimd.memset(bexp[:], float(EXP_BIAS))

            nchunk = M_COLS // psum_fd
            loop_cm = tc.For_i(0, iters) if iters > 1 else contextlib.nullcontext(0)
            with loop_cm as _i:
                zw_tiles = {}
                last_evac = None
                for m in range(MTILES):
                    zw = zwpool.tile([128, M_COLS], F16, tag="zw")
                    zw_tiles[m] = zw
                    msl = slice(m * 128, (m + 1) * 128)
                    for nb in range(nchunk):
                        pt = psum.tile([128, psum_fd], F32, tag="ps")
                        for j in range(psum_fd // mm_n):
                            col = nb * psum_fd + j * mm_n
                            csl = slice(col, col + mm_n)
                            nc.tensor.matmul(
                                pt[:, j * mm_n:(j + 1) * mm_n],
                                xs[:, msl], ys[:, csl],
                                start=True, stop=False,
                            )
                            nc.tensor.matmul(
                                pt[:, j * mm_n:(j + 1) * mm_n],
                                xs[:DIM, msl], yl[:, csl],
                                start=False, stop=True,
                            )
                        sl = slice(nb * psum_fd, (nb + 1) * psum_fd)
                        # w = sqrt(200*c + 200 + delta), written as fp16
                        last_evac = nc.scalar.activation(
                            zw[:, sl], pt[:], AF.Sqrt, bias=bsq[:], scale=200.0
                        )
                for m in range(MTILES):
                    zw = zw_tiles[m]
                    efd = M_COLS // exp_split
                    obf = obfpool.tile([128, M_COLS], BF16, tag="obf")
                    for e in range(exp_split):
                        esl = slice(e * efd, (e + 1) * efd)
                        exp_inst = nc.scalar.activation(
                            obf[:, esl], zw[:, esl], AF.Exp,
                            bias=bexp[:], scale=float(EXP_SCALE)
                        )
                        # keep every Exp behind the last Sqrt evac so the ACT
                        # table is switched exactly twice per iteration
                        add_dep_helper(
                            exp_inst.ins, last_evac.ins, sync=False,
                            reason="batch exps after all sqrts (table switch)",
                        )
                        nc.sync.dma_start(
                            out=out_d[m * 128:(m + 1) * 128, esl], in_=obf[:, esl]
                        )

    nc.finalize()
    return nc


LAST_RESULTS = None


def _split_bf16(a):
    hi = a.astype(BFNP)
    lo = (a - hi.astype(np.float32)).astype(BFNP)
    return hi, lo


def kernel(x: np.ndarray, y: np.ndarray) -> np.ndarray:
    global LAST_RESULTS
    x = np.ascontiguousarray(x, dtype=np.float32)
    y = np.ascontiguousarray(y, dtype=np.float32)
    assert x.shape == (N_ROWS, DIM) and y.shape == (M_COLS, DIM)

    if MODE not in _cache:
        _cache[MODE] = _build(MODE)
    nc = _cache[MODE]

    yT = y.T
    yh, yl = _split_bf16(yT)
    ys = np.ascontiguousarray(np.concatenate([yh, yh], axis=0))
    yl = np.ascontiguousarray(yl)

    in_maps = []
    for i in range(N_CORES):
        xT = x[i * ROWS:(i + 1) * ROWS].T
        xh, xl = _split_bf16(xT)
        xstack = np.ascontiguousarray(np.concatenate([xh, xl], axis=0))
        in_maps.append({"xs": xstack, "ys": ys, "yl": yl})

    LAST_RESULTS = run_bass_kernel_spmd(nc, in_maps, list(range(N_CORES)))
    out = np.concatenate([r["out"] for r in LAST_RESULTS.results], axis=0)
    if out.dtype != np.float32:
        out = out.astype(np.float32)
    return out


# revision 5
# speedup vs baseline: 1.6940x; 1.5076x over previous
"""Trainium2 Bass kernel for nn_Bessel: out = i0e(z) * exp(z - 2a), z = 2a*sqrt((1+x@yT)/2), a=10.

Math: out = exp(z - 20 + ln i0e(z)) = exp(t(z) - 20), t(z) = z + ln i0e(z).
With unit-norm rows, z = sqrt(200*c + 200) for c = x@yT in [-0.726, 0.816],
so z lies in [7.4, 19.1].  Key identity used here ("sqs" mode):

  t(z) ~= A + B*sqrt(z^2 + delta)        max rel err 2.05e-3 on [7.2, 19.3]

Since z^2 = 200c + 200 is linear in c, the whole correction folds into the
Sqrt activation's free affine:  w = Sqrt(200*c + (200+delta)), out =
Exp(B*w + (A-20)).  Exactly two ACT passes per element, no DVE pass.

Matmul: a single fp16 x fp16 matmul (fp16's 11-bit mantissa on unit-norm
data is accurate enough: ~3e-4 out rel err) replaces the bf16 hi/lo split
(which needed 2 matmuls with alternating stationary operands; measured
~467-553 ns per 512-col matmul from per-instruction LDWEIGHTS+dispatch
overhead -> 120-142 us of PE time, the previous bottleneck).  Now: 64
matmuls of K=64, N=1024 with one stationary per M-tile -> ~35 us PE.

Schedule per core (row-shard of x, y replicated; no collectives):
  Phase A (sqrt table): per 128x2048 PSUM chunk (2 bufs): 2x PE fp16
      matmul -> ACT Sqrt evac direct to fp16 w tiles (all 8 M-tiles in SBUF)
  Phase B (exp table):  per M-tile: ACT Exp -> bf16 out tile -> DMA to HBM
  One table switch per phase; bf16 output halves the HBM write (16 MB/core,
  upcast to fp32 on host).

Error budget (numpy-simulated, seed-0 inputs): L2 rel 4.4e-3
(fit 1.2e-3 + fp16 w 3.9e-3 + bf16 out 1.1e-3 + fp16 mm 0.3e-3), gate 2e-2.
ACT busy model: 32 sqrt x ~2.08us + 8 exp x ~7.01us + 2 table loads
  = 66.6 + 56.1 + 2.6 = ~125us steady-state target.
"""

import contextlib

import numpy as np

import concourse.bacc as bacc
import concourse.mybir as mybir
from concourse.tile import TileContext
from concourse.tile_autobufs import add_dep_helper
from concourse.bass_utils import run_bass_kernel_spmd

AF = mybir.ActivationFunctionType
F32 = mybir.dt.float32
F16 = mybir.dt.float16
BF16 = mybir.dt.bfloat16

N_CORES = 8
N_ROWS, M_COLS, DIM = 8192, 8192, 64
ROWS = N_ROWS // N_CORES          # 1024 rows of x per core
MTILES = ROWS // 128              # 8 partition tiles per core
PSUM_FD = 2048                    # 4 PSUM banks per psum tile, 2 bufs
MM_N = 512                        # moving free dim (psum bank limit)

# minimax fit of t(z) = z + ln(i0e(z)) ~= A + B*sqrt(z^2 + delta), z in [7.2, 19.3]
SQS_A = -2.18471144825
SQS_B = 0.980389112036
SQS_D = 6.67629017188
SQ_BIAS = 200.0 + SQS_D           # Sqrt(200*c + SQ_BIAS)
EXP_SCALE = SQS_B
EXP_BIAS = SQS_A - 20.0

MODE = "sqs"

_cache = {}


def _build(mode, iters=1, psum_fd=PSUM_FD, exp_split=1, obf_bufs=2, mm_n=MM_N):
    assert mode == "sqs"
    nc = bacc.Bacc(None, target_bir_lowering=False)
    xq_d = nc.dram_tensor("xq", [DIM, ROWS], F16, kind="ExternalInput")
    yq_d = nc.dram_tensor("yq", [DIM, M_COLS], F16, kind="ExternalInput")
    out_d = nc.dram_tensor("out", [ROWS, M_COLS], BF16, kind="ExternalOutput")

    with TileContext(nc) as tc:
        with (
            tc.tile_pool(name="inp", bufs=1) as inp,
            tc.tile_pool(name="consts", bufs=1) as consts,
            tc.tile_pool(name="zw", bufs=MTILES) as zwpool,
            tc.tile_pool(name="obf", bufs=obf_bufs) as obfpool,
            tc.tile_pool(name="psum", bufs=4096 // psum_fd, space="PSUM") as psum,
        ):
            xq = inp.tile([DIM, ROWS], F16)
            yq = inp.tile([DIM, M_COLS], F16)
            nc.sync.dma_start(out=xq[:], in_=xq_d[:])
            for q in range(0, M_COLS, 2048):
                nc.sync.dma_start(out=yq[:, q:q + 2048], in_=yq_d[:, q:q + 2048])

            bsq = consts.tile([128, 1], F32)
            nc.gpsimd.memset(bsq[:], float(SQ_BIAS))
            bexp = consts.tile([128, 1], F32)
            nc.gpsimd.memset(bexp[:], float(EXP_BIAS))

            nchunk = M_COLS // psum_fd
            loop_cm = tc.For_i(0, iters) if iters > 1 else contextlib.nullcontext(0)
            with loop_cm as _i:
                zw_tiles = {}
                last_evac = None
                for m in range(MTILES):
                    zw = zwpool.tile([128, M_COLS], F16, tag="zw")
                    zw_tiles[m] = zw
                    msl = slice(m * 128, (m + 1) * 128)
                    for nb in range(nchunk):
                        pt = psum.tile([128, psum_fd], F32, tag="ps")
                        for j in range(psum_fd // mm_n):
                            col = nb * psum_fd + j * mm_n
                            nc.tensor.matmul(
                                pt[:, j * mm_n:(j + 1) * mm_n],
                                xq[:, msl], yq[:, col:col + mm_n],
                                start=True, stop=True,
                            )
                        sl = slice(nb * psum_fd, (nb + 1) * psum_fd)
                        # w = sqrt(200*c + 200 + delta), written as fp16
                        last_evac = nc.scalar.activation(
                            zw[:, sl], pt[:], AF.Sqrt, bias=bsq[:], scale=200.0
                        )
                for m in range(MTILES):
                    zw = zw_tiles[m]
                    efd = M_COLS // exp_split
                    obf = obfpool.tile([128, M_COLS], BF16, tag="obf")
                    for e in range(exp_split):
                        esl = slice(e * efd, (e + 1) * efd)
                        exp_inst = nc.scalar.activation(
                            obf[:, esl], zw[:, esl], AF.Exp,
                            bias=bexp[:], scale=float(EXP_SCALE)
                        )
                        # keep every Exp behind the last Sqrt evac so the ACT
                        # table is switched exactly twice per iteration
                        add_dep_helper(
                            exp_inst.ins, last_evac.ins, sync=False,
                            reason="batch exps after all sqrts (table switch)",
                        )
                        nc.sync.dma_start(
                            out=out_d[m * 128:(m + 1) * 128, esl], in_=obf[:, esl]
                        )

    nc.finalize()
    return nc


LAST_RESULTS = None


def _prep_inputs(x, y):
    """FULL fp32 x, y -> per-core input maps (fp16, transposed)."""
    yq = np.ascontiguousarray(y.T.astype(np.float16))
    in_maps = []
    for i in range(N_CORES):
        xq = np.ascontiguousarray(x[i * ROWS:(i + 1) * ROWS].T.astype(np.float16))
        in_maps.append({"xq": xq, "yq": yq})
    return in_maps


def kernel(x: np.ndarray, y: np.ndarray) -> np.ndarray:
    global LAST_RESULTS
    x = np.ascontiguousarray(x, dtype=np.float32)
    y = np.ascontiguousarray(y, dtype=np.float32)
    assert x.shape == (N_ROWS, DIM) and y.shape == (M_COLS, DIM)

    if MODE not in _cache:
        _cache[MODE] = _build(MODE)
    nc = _cache[MODE]

    in_maps = _prep_inputs(x, y)
    LAST_RESULTS = run_bass_kernel_spmd(nc, in_maps, list(range(N_CORES)))
    out = np.concatenate([r["out"] for r in LAST_RESULTS.results], axis=0)
    if out.dtype != np.float32:
        out = out.astype(np.float32)
    return out


# revision 6
# speedup vs baseline: 1.7913x; 1.0574x over previous
"""Trainium2 Bass kernel for nn_Bessel: out = i0e(z) * exp(z - 2a), z = 2a*sqrt((1+x@yT)/2), a=10.

Math: out = exp(z - 20 + ln i0e(z)) = exp(t(z) - 20), t(z) = z + ln i0e(z).
With unit-norm rows, z = sqrt(200*c + 200) for c = x@yT in [-0.726, 0.816],
so z lies in [7.4, 19.1].  Key identity used here ("sqs" mode):

  t(z) ~= A + B*sqrt(z^2 + delta)        max rel err 2.05e-3 on [7.2, 19.3]

Since z^2 = 200c + 200 is linear in c, the whole correction folds into the
Sqrt activation's free affine:  w = Sqrt(200*c + (200+delta)), out =
Exp(B*w + (A-20)).  Exactly two ACT passes per element, no DVE pass.

Matmul: a single fp16 x fp16 matmul (fp16's 11-bit mantissa on unit-norm
data is accurate enough: ~3e-4 out rel err) replaces the bf16 hi/lo split
(which needed 2 matmuls with alternating stationary operands; measured
~467-553 ns per 512-col matmul from per-instruction LDWEIGHTS+dispatch
overhead -> 120-142 us of PE time, the previous bottleneck).  Now: 64
matmuls of K=64, N=1024 with one stationary per M-tile -> ~35 us PE.

Schedule per core (row-shard of x, y replicated; no collectives):
  Phase A (sqrt table): per 128x2048 PSUM chunk (2 bufs): 2x PE fp16
      matmul -> ACT Sqrt evac direct to fp16 w tiles (all 8 M-tiles in SBUF)
  Phase B (exp table):  per M-tile: ACT Exp -> bf16 out tile -> DMA to HBM
  One table switch per phase; bf16 output halves the HBM write (16 MB/core,
  upcast to fp32 on host).

Error budget (numpy-simulated, seed-0 inputs): L2 rel 4.4e-3
(fit 1.2e-3 + fp16 w 3.9e-3 + bf16 out 1.1e-3 + fp16 mm 0.3e-3), gate 2e-2.
ACT busy model: 32 sqrt x ~2.08us + 8 exp x ~7.01us + 2 table loads
  = 66.6 + 56.1 + 2.6 = ~125us steady-state target.
"""

import contextlib

import numpy as np

import concourse.bacc as bacc
import concourse.mybir as mybir
from concourse.tile import TileContext
from concourse.tile_autobufs import add_dep_helper
from concourse.bass_utils import run_bass_kernel_spmd

AF = mybir.ActivationFunctionType
F32 = mybir.dt.float32
F16 = mybir.dt.float16
BF16 = mybir.dt.bfloat16

N_CORES = 8
N_ROWS, M_COLS, DIM = 8192, 8192, 64
ROWS = N_ROWS // N_CORES          # 1024 rows of x per core
MTILES = ROWS // 128              # 8 partition tiles per core
PSUM_FD = 2048                    # 4 PSUM banks per psum tile, 2 bufs
MM_N = 512                        # moving free dim (psum bank limit)

# minimax fit of t(z) = z + ln(i0e(z)) ~= A + B*sqrt(z^2 + delta), z in [7.2, 19.3]
SQS_A = -2.18471144825
SQS_B = 0.980389112036
SQS_D = 6.67629017188
SQ_BIAS = 200.0 + SQS_D           # Sqrt(200*c + SQ_BIAS)
EXP_SCALE = SQS_B
EXP_BIAS = SQS_A - 20.0

MODE = "sqs"

_cache = {}


def _build(mode, iters=1, psum_fd=PSUM_FD, exp_split=1, obf_bufs=3, mm_n=MM_N):
    assert mode == "sqs"
    nc = bacc.Bacc(None, target_bir_lowering=False)
    xq_d = nc.dram_tensor("xq", [DIM, ROWS], F16, kind="ExternalInput")
    yq_d = nc.dram_tensor("yq", [DIM, M_COLS], F16, kind="ExternalInput")
    out_d = nc.dram_tensor("out", [ROWS, M_COLS], BF16, kind="ExternalOutput")

    with TileContext(nc) as tc:
        with (
            tc.tile_pool(name="inp", bufs=1) as inp,
            tc.tile_pool(name="consts", bufs=1) as consts,
            tc.tile_pool(name="zw", bufs=MTILES) as zwpool,
            tc.tile_pool(name="obf", bufs=obf_bufs) as obfpool,
            tc.tile_pool(name="psum", bufs=4096 // psum_fd, space="PSUM") as psum,
        ):
            xq = inp.tile([DIM, ROWS], F16)
            yq = inp.tile([DIM, M_COLS], F16)
            nc.sync.dma_start(out=xq[:], in_=xq_d[:])
            for q in range(0, M_COLS, 2048):
                nc.sync.dma_start(out=yq[:, q:q + 2048], in_=yq_d[:, q:q + 2048])

            bsq = consts.tile([128, 1], F32)
            nc.gpsimd.memset(bsq[:], float(SQ_BIAS))
            bexp = consts.tile([128, 1], F32)
            nc.gpsimd.memset(bexp[:], float(EXP_BIAS))

            nchunk = M_COLS // psum_fd
            loop_cm = tc.For_i(0, iters) if iters > 1 else contextlib.nullcontext(0)
            with loop_cm as _i:
                zw_tiles = {}
                last_evac = None
                for m in range(MTILES):
                    zw = zwpool.tile([128, M_COLS], F16, tag="zw")
                    zw_tiles[m] = zw
                    msl = slice(m * 128, (m + 1) * 128)
                    for nb in range(nchunk):
                        pt = psum.tile([128, psum_fd], F32, tag="ps")
                        for j in range(psum_fd // mm_n):
                            col = nb * psum_fd + j * mm_n
                            nc.tensor.matmul(
                                pt[:, j * mm_n:(j + 1) * mm_n],
                                xq[:, msl], yq[:, col:col + mm_n],
                                start=True, stop=True,
                            )
                        sl = slice(nb * psum_fd, (nb + 1) * psum_fd)
                        # w = sqrt(200*c + 200 + delta), written as fp16
                        last_evac = nc.scalar.activation(
                            zw[:, sl], pt[:], AF.Sqrt, bias=bsq[:], scale=200.0
                        )
                for m in range(MTILES):
                    zw = zw_tiles[m]
                    efd = M_COLS // exp_split
                    obf = obfpool.tile([128, M_COLS], BF16, tag="obf")
                    for e in range(exp_split):
                        esl = slice(e * efd, (e + 1) * efd)
                        exp_inst = nc.scalar.activation(
                            obf[:, esl], zw[:, esl], AF.Exp,
                            bias=bexp[:], scale=float(EXP_SCALE)
                        )
                        # keep every Exp behind the last Sqrt evac so the ACT
                        # table is switched exactly twice per iteration
                        add_dep_helper(
                            exp_inst.ins, last_evac.ins, sync=False,
                            reason="batch exps after all sqrts (table switch)",
                        )
                        nc.sync.dma_start(
                            out=out_d[m * 128:(m + 1) * 128, esl], in_=obf[:, esl]
                        )

    nc.finalize()
    return nc


LAST_RESULTS = None


def _prep_inputs(x, y):
    """FULL fp32 x, y -> per-core input maps (fp16, transposed)."""
    yq = np.ascontiguousarray(y.T.astype(np.float16))
    in_maps = []
    for i in range(N_CORES):
        xq = np.ascontiguousarray(x[i * ROWS:(i + 1) * ROWS].T.astype(np.float16))
        in_maps.append({"xq": xq, "yq": yq})
    return in_maps


def kernel(x: np.ndarray, y: np.ndarray) -> np.ndarray:
    global LAST_RESULTS
    x = np.ascontiguousarray(x, dtype=np.float32)
    y = np.ascontiguousarray(y, dtype=np.float32)
    assert x.shape == (N_ROWS, DIM) and y.shape == (M_COLS, DIM)

    if MODE not in _cache:
        _cache[MODE] = _build(MODE)
    nc = _cache[MODE]

    in_maps = _prep_inputs(x, y)
    LAST_RESULTS = run_bass_kernel_spmd(nc, in_maps, list(range(N_CORES)))
    out = np.concatenate([r["out"] for r in LAST_RESULTS.results], axis=0)
    if out.dtype != np.float32:
        out = out.astype(np.float32)
    return out
